# revision 9
# baseline (speedup 1.0000x reference)
"""ROIPooling (adaptive 7x7 max pool over per-ROI valid h x w regions) on 8 trn2 cores.

Strategy (data-parallel over ROI dim N, 64 ROIs per core):
  - SBUF layout: partition p = (roi n, channel-half ch); free = (c 128, r 14, w 14).
  - Adaptive bins [floor(i*L/7), ceil((i+1)*L/7)) have width 1..3 and start in
    [i, 2i]; all candidate rows for bin i lie in [i, i + WPAD[i]).
    Pooling = running max over candidate slots of (x + mask) where
    mask[p, i, rr] = 0 inside the ROI's window, -3e38 outside.  Masks are
    built on host from h/w (tiny aux tensors) -> one shared SPMD program.
  - Each candidate slot is ONE fused scalar_tensor_tensor op:
        acc = max(x[:, c, i+rr, :] + vmask[p, islot], acc)
    (op0=add with per-partition scalar, op1=max accumulating in place).
    First slot uses op1=bypass, so no init memset is needed.
  - Channels are split between DVE (nc.vector) and GpSimd (nc.gpsimd); DMA is
    dense HWDGE (nc.sync).  Vertical then horizontal stage.
"""

import numpy as np
from contextlib import ExitStack

import concourse.bass as bass
import concourse.bacc as bacc
import concourse.tile as tile
from concourse import mybir
from concourse.bass_utils import run_bass_kernel_spmd

N, C, H, W, OUT = 512, 256, 14, 14, 7
NCORES = 8
NS = N // NCORES          # ROIs per core
CH = C // 2               # channels per partition (2 partitions per ROI)
NEG = -3.0e38
WPAD = [2, 3, 4, 5, 6, 7, 8]   # candidate-slot count per output bin
CD = 56                   # channels on DVE; rest on GpSimd
CG = CH - CD

FP32 = mybir.dt.float32


def _bins(L):
    i = np.arange(OUT)
    s = (i * L) // OUT
    e = ((i + 1) * L + OUT - 1) // OUT
    return s, e


def build_masks(sizes):
    """sizes: [NS] valid lengths -> [128, OUT, 8] additive mask (0 or NEG)."""
    m = np.full((NS, OUT, 8), NEG, np.float32)
    for n, L in enumerate(sizes):
        s, e = _bins(int(L))
        for i in range(OUT):
            for rr in range(WPAD[i]):
                r = i + rr
                if s[i] <= r < e[i]:
                    m[n, i, rr] = 0.0
    return np.repeat(m, 2, axis=0)  # partition p = (n, ch)


def build_program(cd=CD):
    cg = CH - cd
    nc = bacc.Bacc("TRN2", target_bir_lowering=False, debug=False,
                   num_devices=NCORES)
    x = nc.dram_tensor("x", [128, CH, H, W], FP32, kind="ExternalInput").ap()
    vm = nc.dram_tensor("vm", [128, OUT, 8], FP32, kind="ExternalInput").ap()
    hm = nc.dram_tensor("hm", [128, OUT, 8], FP32, kind="ExternalInput").ap()
    out = nc.dram_tensor("out", [128, CH, OUT, OUT], FP32,
                         kind="ExternalOutput").ap()

    ADD = mybir.AluOpType.add
    MAX = mybir.AluOpType.max
    BYP = mybir.AluOpType.bypass

    with tile.TileContext(nc) as tc, ExitStack() as ctx:
        singles = ctx.enter_context(tc.tile_pool(name="singles", bufs=1))
        big = ctx.enter_context(tc.tile_pool(name="big", bufs=1))

        vm_t = singles.tile([128, OUT * 8], FP32)
        nc.scalar.dma_start(vm_t[:], vm.rearrange("p a b -> p (a b)"))
        hm_t = singles.tile([128, OUT * 8], FP32)
        nc.scalar.dma_start(hm_t[:], hm.rearrange("p a b -> p (a b)"))

        # All slot ops run on DVE: TRN2 walrus rejects TensorScalarPtr and
        # TensorTensor on Pool, and ACT cannot max.  Chunked contiguous
        # loads let compute overlap the remaining DMA.
        engines = [
            (nc.vector, 0, 44),
            (nc.vector, 44, 44),
            (nc.vector, 88, CH - 88),
        ]

        x_t, rowp, out_t = {}, {}, {}
        for eng, c0, ncc in engines:
            x_t[c0] = big.tile([128, ncc, H, W], FP32, tag=f"x{c0}", name=f"x{c0}")
            nc.sync.dma_start(x_t[c0][:], x[:, c0:c0 + ncc])
            rowp[c0] = big.tile([128, ncc, W, OUT], FP32, tag=f"rp{c0}", name=f"rp{c0}")
            out_t[c0] = big.tile([128, ncc, OUT, OUT], FP32, tag=f"ot{c0}", name=f"ot{c0}")

        # vertical: rowp[c, w, i] = max_rr(x[c, i+rr, w] + vm[p, i*8+rr])
        for eng, c0, ncc in engines:
            xe, rpe = x_t[c0], rowp[c0]
            for i in range(OUT):
                acc = rpe[:, :, :, i]
                for rr in range(WPAD[i]):
                    src = xe[:, :, i + rr, :]
                    eng.scalar_tensor_tensor(
                        out=acc, in0=src,
                        scalar=vm_t[:, i * 8 + rr: i * 8 + rr + 1],
                        in1=(src if rr == 0 else acc),
                        op0=ADD, op1=(BYP if rr == 0 else MAX))

        # horizontal: out[c, i, j] = max_wr(rowp[c, j+wr, i] + hm[p, j*8+wr])
        for eng, c0, ncc in engines:
            rpe, ote = rowp[c0], out_t[c0]
            for j in range(OUT):
                acc = ote[:, :, :, j]
                for wr in range(WPAD[j]):
                    src = rpe[:, :, j + wr, :]
                    eng.scalar_tensor_tensor(
                        out=acc, in0=src,
                        scalar=hm_t[:, j * 8 + wr: j * 8 + wr + 1],
                        in1=(src if wr == 0 else acc),
                        op0=ADD, op1=(BYP if wr == 0 else MAX))

        for eng, c0, ncc in engines:
            nc.scalar.dma_start(out[:, c0:c0 + ncc], out_t[c0][:])

        del x_t, rowp, out_t

    nc.compile()
    return nc


def make_in_maps(rois, h, w):
    rois = np.ascontiguousarray(rois, np.float32).reshape(N, C, H, W)
    h = np.asarray(h).astype(np.int64)
    w = np.asarray(w).astype(np.int64)
    in_maps = []
    for k in range(NCORES):
        sl = slice(k * NS, (k + 1) * NS)
        xk = rois[sl].reshape(128, CH, H, W)  # p=(n,ch) nests exactly
        in_maps.append({
            "x": xk,
            "vm": build_masks(h[sl]),
            "hm": build_masks(w[sl]),
        })
    return in_maps


_PROG = None


def kernel(rois, h, w):
    global _PROG
    if _PROG is None:
        _PROG = build_program()
    in_maps = make_in_maps(rois, h, w)
    res = run_bass_kernel_spmd(_PROG, in_maps, list(range(NCORES)))
    outs = [res.results[k]["out"].reshape(NS * C, OUT, OUT)
            for k in range(NCORES)]
    return np.concatenate(outs, axis=0)


# revision 11
# speedup vs baseline: 1.0404x; 1.0404x over previous
"""ROIPooling (adaptive 7x7 max pool over per-ROI valid h x w regions) on 8 trn2 cores.

Strategy (data-parallel over ROI dim N, 64 ROIs per core):
  - SBUF layout: partition p = (roi n, channel-half ch); free = (c 128, r 14, w 14).
  - Adaptive bins [floor(i*L/7), ceil((i+1)*L/7)) have width 1..3 and start in
    [i, 2i]; all candidate rows for bin i lie in [i, i + WPAD[i]).
    Pooling = running max over candidate slots of (x + mask) where
    mask[p, i, rr] = 0 inside the ROI's window, -3e38 outside.  Masks are
    built on host from h/w (tiny aux tensors) -> one shared SPMD program.
  - Each candidate slot is ONE fused scalar_tensor_tensor op:
        acc = max(x[:, c, i+rr, :] + vmask[p, islot], acc)
    (op0=add with per-partition scalar, op1=max accumulating in place).
    First slot uses op1=bypass, so no init memset is needed.
  - Channels are split between DVE (nc.vector) and GpSimd (nc.gpsimd); DMA is
    dense HWDGE (nc.sync).  Vertical then horizontal stage.
"""

import numpy as np
from contextlib import ExitStack

import concourse.bass as bass
import concourse.bacc as bacc
import concourse.tile as tile
from concourse import mybir
from concourse.bass_utils import run_bass_kernel_spmd

N, C, H, W, OUT = 512, 256, 14, 14, 7
NCORES = 8
NS = N // NCORES          # ROIs per core
CH = C // 2               # channels per partition (2 partitions per ROI)
NEG = -3.0e38
WPAD = [2, 3, 4, 5, 6, 7, 8]   # candidate-slot count per output bin
CD = 56                   # channels on DVE; rest on GpSimd
CG = CH - CD

FP32 = mybir.dt.float32


def _bins(L):
    i = np.arange(OUT)
    s = (i * L) // OUT
    e = ((i + 1) * L + OUT - 1) // OUT
    return s, e


def build_masks(sizes):
    """sizes: [NS] valid lengths -> [128, OUT, 8] additive mask (0 or NEG)."""
    m = np.full((NS, OUT, 8), NEG, np.float32)
    for n, L in enumerate(sizes):
        s, e = _bins(int(L))
        for i in range(OUT):
            for rr in range(WPAD[i]):
                r = i + rr
                if s[i] <= r < e[i]:
                    m[n, i, rr] = 0.0
    return np.repeat(m, 2, axis=0)  # partition p = (n, ch)


def build_program(cd=CD):
    cg = CH - cd
    nc = bacc.Bacc("TRN2", target_bir_lowering=False, debug=False,
                   num_devices=NCORES)
    x = nc.dram_tensor("x", [128, CH, H, W], FP32, kind="ExternalInput").ap()
    vm = nc.dram_tensor("vm", [128, OUT, 8], FP32, kind="ExternalInput").ap()
    hm = nc.dram_tensor("hm", [128, OUT, 8], FP32, kind="ExternalInput").ap()
    out = nc.dram_tensor("out", [128, CH, OUT, OUT], FP32,
                         kind="ExternalOutput").ap()

    ADD = mybir.AluOpType.add
    MAX = mybir.AluOpType.max
    BYP = mybir.AluOpType.bypass

    with tile.TileContext(nc) as tc, ExitStack() as ctx:
        singles = ctx.enter_context(tc.tile_pool(name="singles", bufs=1))
        big = ctx.enter_context(tc.tile_pool(name="big", bufs=1))

        vm_t = singles.tile([128, OUT * 8], FP32)
        nc.scalar.dma_start(vm_t[:], vm.rearrange("p a b -> p (a b)"))
        hm_t = singles.tile([128, OUT * 8], FP32)
        nc.scalar.dma_start(hm_t[:], hm.rearrange("p a b -> p (a b)"))

        # All slot ops run on DVE: TRN2 walrus rejects TensorScalarPtr and
        # TensorTensor on Pool, and ACT cannot max.  Chunked contiguous
        # loads let compute overlap the remaining DMA; the first chunk is
        # small so DVE starts early.
        engines = [
            (nc.vector, 0, 24),
            (nc.vector, 24, 52),
            (nc.vector, 76, CH - 76),
        ]

        x_t, rowp, out_t = {}, {}, {}
        for eng, c0, ncc in engines:
            x_t[c0] = big.tile([128, ncc, H, W], FP32, tag=f"x{c0}", name=f"x{c0}")
            nc.sync.dma_start(x_t[c0][:], x[:, c0:c0 + ncc])
            rowp[c0] = big.tile([128, ncc, W, OUT], FP32, tag=f"rp{c0}", name=f"rp{c0}")
            out_t[c0] = big.tile([128, ncc, OUT, OUT], FP32, tag=f"ot{c0}", name=f"ot{c0}")

        # per-chunk pipeline: vertical STT chains, horizontal STT chains,
        # then that chunk's output DMA -- later chunks overlap earlier DMAs.
        # vertical: rowp[c, w, i] = max_rr(x[c, i+rr, w] + vm[p, i*8+rr])
        # horizontal: out[c, i, j] = max_wr(rowp[c, j+wr, i] + hm[p, j*8+wr])
        # slot emission is interleaved across bins (rr-major) so consecutive
        # DVE ops hit different accumulators -- no dependent back-to-back ops
        def emit_stage(eng, src_of, acc_of, mask_t):
            for rr in range(max(WPAD)):
                for b in range(OUT):
                    if rr >= WPAD[b]:
                        continue
                    src = src_of(b, rr)
                    acc = acc_of(b)
                    eng.scalar_tensor_tensor(
                        out=acc, in0=src,
                        scalar=mask_t[:, b * 8 + rr: b * 8 + rr + 1],
                        in1=(src if rr == 0 else acc),
                        op0=ADD, op1=(BYP if rr == 0 else MAX))

        for eng, c0, ncc in engines:
            xe, rpe, ote = x_t[c0], rowp[c0], out_t[c0]
            emit_stage(eng, lambda b, rr: xe[:, :, b + rr, :],
                       lambda b: rpe[:, :, :, b], vm_t)
            emit_stage(eng, lambda b, rr: rpe[:, :, b + rr, :],
                       lambda b: ote[:, :, :, b], hm_t)
            nc.scalar.dma_start(out[:, c0:c0 + ncc], ote[:])

        del x_t, rowp, out_t

    nc.compile()
    return nc


def make_in_maps(rois, h, w):
    rois = np.ascontiguousarray(rois, np.float32).reshape(N, C, H, W)
    h = np.asarray(h).astype(np.int64)
    w = np.asarray(w).astype(np.int64)
    in_maps = []
    for k in range(NCORES):
        sl = slice(k * NS, (k + 1) * NS)
        xk = rois[sl].reshape(128, CH, H, W)  # p=(n,ch) nests exactly
        in_maps.append({
            "x": xk,
            "vm": build_masks(h[sl]),
            "hm": build_masks(w[sl]),
        })
    return in_maps


_PROG = None


def kernel(rois, h, w):
    global _PROG
    if _PROG is None:
        _PROG = build_program()
    in_maps = make_in_maps(rois, h, w)
    res = run_bass_kernel_spmd(_PROG, in_maps, list(range(NCORES)))
    outs = [res.results[k]["out"].reshape(NS * C, OUT, OUT)
            for k in range(NCORES)]
    return np.concatenate(outs, axis=0)
